# revision 31
# baseline (speedup 1.0000x reference)
"""Multi-head self-attention (B=4, N=2048, C=768, H=12, causal + RoPE) on 8 TRN2 cores.

Sharding: core = (batch b = core // 2, head-group g = core % 2); each core computes
6 heads of one batch end-to-end (qkv -> rope -> causal flash attention -> partial
output projection over its 384 channels). Host sums the two partial projections
per batch and adds the bias.

Device layout notes:
  - everything is kept "transposed" ([channel, token]); attention scores are
    computed directly as scoresT[k, q] = kT' . qT', PV needs no transposes.
  - phases are software-pipelined at every level: qkv for block b+1 and the
    output projection for block b-1 are emitted inside attention block b
    (after head-pair 1), exp runs TWO k-tiles ahead of PV, and each head
    pair's normalize is deferred into the next head pair's score stream.
  - qkv PSUM evacuation runs on the Act engine (idle during qkv); rope is
    2 muls + add on bf16 SBUF tiles, d-tile 0 on DVE (needed first), d-tiles
    1-2 on the otherwise-idle gpsimd.  swap32 is 4 partition-crossing SBUF
    DMAs per d-tile.
  - V carries an extra all-ones column per head; the PV matmul then
    accumulates the softmax denominator in psum row 64 for free.
  - causal trimming: on diagonal-band k-tiles only the valid q-suffix is
    computed; the in-band triangle is zeroed by a constant 0/1 bf16 mask
    multiply on DVE (NOT gpsimd affine_select: Pool dispatch latency of
    3-6us stalls the PV pipeline).
  - normalize: den rows copied to SBUF partitions 0 and 64 (engine start
    partitions must be 64-aligned), ONE K=65 selector matmul broadcasts
    1/denA to psum partitions 0-63 and 1/denB to 64-127, one reciprocal,
    then one stt per head writing attnT[0:64]/[64:128] directly (DVE
    operands may have different 64-aligned base partitions, so head B
    needs no shift DMA).  No gpsimd anywhere near the critical path.
  - input DMAs are split across the SP and Act hwdge queues (~85-150GB/s
    per queue), most-urgent (wq, x block 0) first; output DMAs of the last
    block alternate queues for the same reason.
  - PSUM budget: 2 banks qkv/V/proj/bcast ring + 4 banks score supers
    (double buffered) + 2 banks PV accumulators = 8.
"""

import os
import sys

import numpy as np

sys.path.insert(0, "/opt/trn_rl_repo")

import concourse.bass as bass
import concourse.mybir as mybir
import concourse.tile as tile
from concourse import bacc
from concourse.bass_utils import run_bass_kernel_spmd

dt = mybir.dt
F32 = dt.float32
BF16 = dt.bfloat16
AF = mybir.ActivationFunctionType
ALU = mybir.AluOpType

B, N, C = 4, 2048, 768
H, HD = 12, 64
HL = 6            # heads per core
G = 2             # head groups (cores per batch)
NCORES = 8
NT = N // 128     # 16 n-tiles
QB = 512          # query block
NQB = N // QB     # 4 query blocks
CT = C // 128     # 6 contraction tiles of x channels
DL = HL * HD      # 384 local channels
VW = HL * (HD + 1)  # 390: v columns + ones column per head
SCALE = float(HD) ** -0.5

LAST_RESULTS = None  # test harness can read exec_time_ns etc. from here


def _emit(nc, tc, dram):
    xT_d, wqT_d, wkT_d, wvaT_d, cosT2_d, sinT2t_d, projwT_d, outp_d = dram

    with tc.tile_pool(name="persist", bufs=1) as pp:
        qT = [pp.tile([128, N], BF16, tag=f"qT{t}", name=f"qT{t}") for t in range(3)]
        kT = [pp.tile([128, N], BF16, tag=f"kT{t}", name=f"kT{t}") for t in range(3)]
        attnT = [pp.tile([128, N], BF16, tag=f"aT{t}", name=f"aT{t}") for t in range(3)]
        V = [pp.tile([128, VW], BF16, tag=f"V{t}", name=f"V{t}") for t in range(NT)]

        wq = pp.tile([128, CT * DL], BF16, tag="wq", name="wq")
        wk = pp.tile([128, CT * DL], BF16, tag="wk", name="wk")
        wva = pp.tile([128, CT * VW], BF16, tag="wva", name="wva")
        cosT2 = pp.tile([128, N], BF16, tag="cosT2", name="cosT2")
        sinT2t = pp.tile([128, N], BF16, tag="sinT2t", name="sinT2t")
        pw = [pp.tile([128, C], BF16, tag=f"pw{t}", name=f"pw{t}") for t in range(3)]

        with (
            tc.tile_pool(name="mm_ps", bufs=2, space="PSUM") as mmp,
            tc.tile_pool(name="score_ps", bufs=2, space="PSUM") as scp,
            tc.tile_pool(name="out_ps", bufs=2, space="PSUM") as abp,
            tc.tile_pool(name="work", bufs=1) as ap,
        ):
            xtbs = {}
            pending = []  # deferred per-head-pair normalize closures

            def flush_pending():
                for fn in pending:
                    fn()
                pending.clear()

            def xtb_dma(nb):
                # two DMAs (SP + Act queues) load the 6 contraction tiles of
                # block nb: xtb[:, QB*ct + j] = x[token QB*nb+j, chan 128ct+p]
                xtb = ap.tile([128, CT * QB], BF16, tag="xtb", bufs=2, name=f"xtb{nb}")
                xv_ = xT_d[:].rearrange("p (c n) -> p c n", c=CT)
                nsl_ = slice(QB * nb, QB * (nb + 1))
                nc.sync.dma_start(xtb[:, 0 : 3 * QB], xv_[:, 0:3, nsl_])
                nc.scalar.dma_start(xtb[:, 3 * QB :], xv_[:, 3:6, nsl_])
                xtbs[nb] = xtb

            # --- preamble DMAs: wq/xtb0 halves split across SP+Act queues ---
            hw_ = CT * DL // 2
            nc.sync.dma_start(wq[:, 0:hw_], wqT_d[:, 0:hw_])
            nc.scalar.dma_start(wq[:, hw_:], wqT_d[:, hw_:])
            xtb_dma(0)
            nc.sync.dma_start(wk[:, 0:hw_], wkT_d[:, 0:hw_])
            nc.scalar.dma_start(wk[:, hw_:], wkT_d[:, hw_:])
            nc.sync.dma_start(cosT2[:], cosT2_d[:])
            nc.sync.dma_start(sinT2t[:], sinT2t_d[:])
            nc.sync.dma_start(wva[:], wvaT_d[:])
            for t in range(3):
                nc.sync.dma_start(pw[t][:], projwT_d[128 * t : 128 * (t + 1), :])
            # [65,128] selector for the one-matmul 1/den broadcast: row 0
            # routes denA to psum partitions 0-63, row 64 routes denB to
            # 64-127 (engine start-partitions must be 64-aligned, so denB
            # lives on partition 64; rows 1-63 are zero selectors).
            ones2 = pp.tile([65, 128], BF16, tag="ones2", name="ones2")
            nc.gpsimd.memset(ones2[:], 0.0)
            nc.gpsimd.memset(ones2[0:1, 0:64], 1.0)
            nc.gpsimd.memset(ones2[64:65, 64:128], 1.0)
            # pre-fill the den2 ring so its unused rows 1-63 are finite
            for i_ in range(3):
                d_ = ap.tile([65, QB], BF16, tag="den2", bufs=3, name=f"dninit{i_}")
                nc.gpsimd.memset(d_[:], 1.0)
            # constant 0/1 causal-triangle mask, duplicated for both heads:
            # tri2[p, 128h + j] = 1.0 if j >= p else 0.0
            tri2 = pp.tile([128, 256], BF16, tag="tri2", name="tri2")
            nc.gpsimd.memset(tri2[:], 1.0)
            nc.gpsimd.affine_select(
                out=tri2[:].rearrange("p (h f) -> p h f", h=2),
                in_=tri2[:].rearrange("p (h f) -> p h f", h=2),
                compare_op=ALU.is_ge,
                fill=0.0,
                base=0,
                pattern=[[0, 2], [1, 128]],
                channel_multiplier=-1,
            )

            def qkv_block(nb):
                nsl = slice(QB * nb, QB * (nb + 1))
                if nb not in xtbs:
                    xtb_dma(nb)
                xtb = xtbs[nb]
                if nb + 1 < NQB:
                    # prefetch next block's x right away (ring has room)
                    xtb_dma(nb + 1)
                qraw = ap.tile([128, 6 * QB], BF16, tag="qraw", bufs=2, name=f"qr{nb}")
                qswp = ap.tile([128, 6 * QB], BF16, tag="qswp", bufs=2, name=f"qs{nb}")
                # q/k matmuls, d-tile-major so pair 0's q and k arrive first;
                # PSUM evacuation on the (idle-during-qkv) Act engine
                for dtile in range(3):
                    for mi, (w, dest) in enumerate(((wq, qT), (wk, kT))):
                        ps = mmp.tile(
                            [128, QB], F32, tag="mm", name=f"ps_qk{nb}_{dtile}{mi}"
                        )
                        for ct in range(CT):
                            nc.tensor.matmul(
                                ps[:],
                                w[:, DL * ct + 128 * dtile : DL * ct + 128 * (dtile + 1)],
                                xtb[:, QB * ct : QB * (ct + 1)],
                                start=(ct == 0),
                                stop=(ct == CT - 1),
                            )
                        wcol = QB * (2 * dtile + mi)
                        nc.scalar.copy(qraw[:, wcol : wcol + QB], ps[:])
                    # swap32 for this dtile's q+k in 4 DMAs:
                    # qswp[64a+32b+c] = qraw[64a+32(1-b)+c]
                    dcol = QB * 2 * dtile
                    for blk in range(4):
                        lo = 32 * blk
                        swp = 32 * (blk + 1) if blk % 2 == 0 else 32 * (blk - 1)
                        nc.sync.dma_start(
                            qswp[lo : lo + 32, dcol : dcol + 2 * QB],
                            qraw[swp : swp + 32, dcol : dcol + 2 * QB],
                        )
                # rope: dst = qraw*cos + swap32(qraw)*sin_tau.  dtile 0 is
                # needed first (head pair 0) -> DVE; dtiles 1-2 have a full
                # head-pair of slack -> idle gpsimd.
                for dtile in range(3):
                    eng = nc.vector if dtile == 0 else nc.gpsimd
                    for mi, dest in enumerate((qT, kT)):
                        wcol = QB * (2 * dtile + mi)
                        dst = dest[dtile][:, nsl]
                        eng.tensor_mul(dst, qraw[:, wcol : wcol + QB], cosT2[:, nsl])
                        tmp = ap.tile(
                            [128, QB], BF16, tag="rtmp", bufs=4, name=f"rt{nb}_{dtile}{mi}"
                        )
                        eng.tensor_mul(
                            tmp[:], qswp[:, wcol : wcol + QB], sinT2t[:, nsl]
                        )
                        eng.tensor_add(dst, dst, tmp[:])
                # V for the 4 n-tiles of this block
                for sub in range(4):
                    nt = 4 * nb + sub
                    ps = mmp.tile([128, VW], F32, tag="mm", name=f"ps_v{nt}")
                    for ct in range(CT):
                        nc.tensor.matmul(
                            ps[:],
                            xtb[:, QB * ct + 128 * sub : QB * ct + 128 * (sub + 1)],
                            wva[:, VW * ct : VW * (ct + 1)],
                            start=(ct == 0),
                            stop=(ct == CT - 1),
                        )
                    nc.vector.tensor_copy(V[nt][:], ps[:])
                    ones_cols = V[nt][:].rearrange("p (h w) -> p h w", w=HD + 1)[
                        :, :, HD : HD + 1
                    ]
                    nc.gpsimd.memset(ones_cols, 1.0)

            def attention_block(qb):
                for pt in range(3):  # head pair (local heads 2pt, 2pt+1)
                    qsl = slice(QB * qb, QB * (qb + 1))
                    nkt = 4 * qb + 4  # causal: k-tiles 0 .. 4qb+3
                    psA = abp.tile([65, QB], F32, tag="outps", name=f"psA{pt}_{qb}")
                    psB = abp.tile([65, QB], F32, tag="outps", name=f"psB{pt}_{qb}")

                    def s_exp(kt):
                        # scoresT[k, q]: head A cols aa:512, B 512+aa:1024
                        ksl = slice(128 * kt, 128 * (kt + 1))
                        a = 128 * kt - QB * qb
                        aa = max(a, 0)
                        S = scp.tile(
                            [128, 2 * QB], F32, tag="sc", name=f"S{pt}_{qb}_{kt}"
                        )
                        for hh in range(2):
                            prow = slice(64 * hh, 64 * hh + 64)
                            nc.tensor.matmul(
                                S[:, QB * hh + aa : QB * (hh + 1)],
                                kT[pt][prow, ksl],
                                qT[pt][prow, QB * qb + aa : QB * (qb + 1)],
                                start=True,
                                stop=True,
                            )
                        P = ap.tile(
                            [128, 2 * QB], BF16, tag="probs", bufs=8,
                            name=f"P{pt}_{qb}_{kt}",
                        )
                        if a > 0:
                            Sv = S[:].rearrange("p (h q) -> p h q", h=2)[:, :, a:QB]
                            Pv = P[:].rearrange("p (h q) -> p h q", h=2)[:, :, a:QB]
                            nc.scalar.activation(Pv, Sv, AF.Exp, scale=SCALE)
                        else:
                            nc.scalar.activation(P[:], S[:], AF.Exp, scale=SCALE)
                        if a >= 0:
                            # zero the causal triangle of the diagonal band
                            # with a constant 0/1 mask on DVE (gpsimd dispatch
                            # latency would stall the PV pipeline)
                            Pband = P[:].rearrange("p (h q) -> p h q", h=2)[
                                :, :, a : a + 128
                            ]
                            nc.vector.tensor_mul(
                                Pband,
                                Pband,
                                tri2[:].rearrange("p (h f) -> p h f", h=2),
                            )
                        return P

                    def pv(kt, P):
                        aa = max(128 * kt - QB * qb, 0)
                        first, last = kt == 0, kt == nkt - 1
                        for hh, ps_out, h in ((0, psA, 2 * pt), (1, psB, 2 * pt + 1)):
                            nc.tensor.matmul(
                                ps_out[:, aa:QB],
                                V[kt][:, 65 * h : 65 * h + 65],
                                P[:, QB * hh + aa : QB * (hh + 1)],
                                start=first,
                                stop=last,
                                skip_group_check=True,
                            )

                    # software-pipelined: S/exp run TWO k-tiles ahead of PV so
                    # the Act engine's exp latency never stalls the PE
                    fifo = [s_exp(0), s_exp(1)]
                    flush_pending()
                    for kt in range(2, nkt):
                        fifo.append(s_exp(kt))
                        pv(kt - 2, fifo.pop(0))
                    pv(nkt - 2, fifo.pop(0))
                    pv(nkt - 1, fifo.pop(0))

                    # ---- normalize epilogue (decoupled from PSUM asap) ----
                    # No gpsimd here: the Pool engine's dispatch latency
                    # (3-6us) was stalling PV/proj.  1/den is broadcast to 64
                    # partitions by a tiny f32r rank-1 matmul instead.
                    rawA = ap.tile([64, QB], BF16, tag="rawA", bufs=3, name=f"rA{pt}_{qb}")
                    rawB = ap.tile([128, QB], BF16, tag="rawB", bufs=3, name=f"rB{pt}_{qb}")
                    den2 = ap.tile(
                        [65, QB], BF16, tag="den2", bufs=3, name=f"dn{pt}_{qb}"
                    )
                    nc.vector.tensor_copy(den2[0:1, :], psA[64:65, :])
                    nc.vector.tensor_copy(den2[64:65, :], psB[64:65, :])
                    nc.vector.tensor_copy(rawA[:], psA[0:64, :])
                    # head B raw goes straight to partitions 64-127 so the
                    # normalize can write attnT[64:128] without a shift DMA
                    nc.vector.tensor_copy(rawB[64:128, :], psB[0:64, :])

                    def normalize(pt=pt, qb=qb, qsl=qsl, rawA=rawA, rawB=rawB, den2=den2):
                        rbc = ap.tile(
                            [128, QB], F32, tag="rbc", bufs=2, name=f"rb{pt}_{qb}"
                        )
                        rb_ps = mmp.tile(
                            [128, QB], F32, tag="mm", name=f"rbp{pt}_{qb}"
                        )
                        nc.tensor.matmul(
                            rb_ps[:], ones2[:], den2[:], start=True, stop=True
                        )
                        nc.vector.reciprocal_approx_fast(rbc[:], rb_ps[:])
                        nc.vector.scalar_tensor_tensor(
                            out=attnT[pt][0:64, qsl],
                            in0=rawA[:],
                            scalar=1.0,
                            in1=rbc[0:64, :],
                            op0=ALU.mult,
                            op1=ALU.mult,
                        )
                        nc.vector.scalar_tensor_tensor(
                            out=attnT[pt][64:128, qsl],
                            in0=rawB[64:128, :],
                            scalar=1.0,
                            in1=rbc[64:128, :],
                            op0=ALU.mult,
                            op1=ALU.mult,
                        )

                    pending.append(normalize)

                    if pt == 1:
                        # overlap next block's qkv and previous block's proj
                        # with the rest of this attention block
                        if qb + 1 < NQB:
                            qkv_block(qb + 1)
                        if qb > 0:
                            proj_block(qb - 1)

            def proj_block(b):
                for nt in range(4 * b, 4 * b + 4):
                    nsl = slice(128 * nt, 128 * (nt + 1))
                    osb = ap.tile([128, C], BF16, tag="osb", bufs=2, name=f"osb{nt}")
                    for half in range(2):
                        ps = mmp.tile([128, 384], F32, tag="mm", name=f"pj{half}_{nt}")
                        for ct in range(3):
                            nc.tensor.matmul(
                                ps[:],
                                attnT[ct][:, nsl],
                                pw[ct][:, 384 * half : 384 * (half + 1)],
                                start=(ct == 0),
                                stop=(ct == 2),
                                skip_group_check=True,
                            )
                        nc.vector.tensor_copy(
                            osb[:, 384 * half : 384 * (half + 1)], ps[:]
                        )
                    if b == NQB - 1 and nt % 2 == 1:
                        nc.scalar.dma_start(outp_d[nsl, :], osb[:])
                    else:
                        nc.sync.dma_start(outp_d[nsl, :], osb[:])

            qkv_block(0)
            for blk in range(NQB):
                attention_block(blk)
            flush_pending()
            proj_block(NQB - 1)


def _build_program():
    nc = bacc.Bacc(
        "TRN2",
        target_bir_lowering=False,
        debug=False,
        num_devices=NCORES,
    )

    dram = (
        nc.dram_tensor("xT", [128, CT * N], BF16, kind="ExternalInput"),
        nc.dram_tensor("wqT", [128, CT * DL], BF16, kind="ExternalInput"),
        nc.dram_tensor("wkT", [128, CT * DL], BF16, kind="ExternalInput"),
        nc.dram_tensor("wvaT", [128, CT * VW], BF16, kind="ExternalInput"),
        nc.dram_tensor("cosT2", [128, N], BF16, kind="ExternalInput"),
        nc.dram_tensor("sinT2t", [128, N], BF16, kind="ExternalInput"),
        nc.dram_tensor("projwT", [DL, C], BF16, kind="ExternalInput"),
        nc.dram_tensor("outp", [N, C], BF16, kind="ExternalOutput"),
    )

    with tile.TileContext(nc) as tc:
        _emit(nc, tc, dram)

    nc.compile()
    return nc


def _rope_tables():
    # mirror reference.rope_tables in float32 (keep the f32 product!)
    inv_freq = 1.0 / np.power(
        np.float32(10000.0), np.arange(0, HD, 2, dtype=np.float32) / np.float32(HD)
    )
    t = np.arange(N, dtype=np.float32)
    freqs = (t[:, None] * inv_freq[None, :].astype(np.float32)).astype(np.float32)
    emb = np.concatenate([freqs, freqs], axis=-1)  # [N, 64]
    return np.cos(emb).astype(np.float32), np.sin(emb).astype(np.float32)


def _make_in_maps(x, qkv_w, proj_w):
    import ml_dtypes

    bf16 = ml_dtypes.bfloat16
    cos, sin = _rope_tables()  # [N, 64]
    # cosT2[p, n] = cos[n, p % 64]
    dd = np.arange(128) % HD
    cosT2 = np.ascontiguousarray(cos.T[dd, :]).astype(bf16)  # [128, N]
    # sin_tau sign such that q' = q*cos + swap32(q)*sin_tau:
    # tau(d) = -1 for d%64 < 32, +1 otherwise  (swap happens BEFORE the
    # sin multiply now, so the sign sits at the destination index)
    sgn = np.where((dd % HD) < (HD // 2), np.float32(-1.0), np.float32(1.0))
    sinT2t = np.ascontiguousarray(sin.T[dd, :] * sgn[:, None]).astype(bf16)

    def pack(a):  # [768, w] -> [128, 6*w]: out[p, w*c + j] = a[128c + p, j]
        w = a.shape[1]
        return np.ascontiguousarray(
            a.reshape(CT, 128, w).transpose(1, 0, 2).reshape(128, CT * w)
        )

    in_maps = []
    for core in range(NCORES):
        b, g = core // G, core % G
        heads = [g * HL + j for j in range(HL)]
        cols = np.concatenate([np.arange(HD * h, HD * h + HD) for h in heads])
        xT = pack(np.ascontiguousarray(x[b].T)).astype(bf16)
        wqT = pack(np.ascontiguousarray(qkv_w[cols, :].T)).astype(bf16)
        wkT = pack(np.ascontiguousarray(qkv_w[C + cols, :].T)).astype(bf16)
        wv = qkv_w[2 * C + cols, :]  # [384, 768]
        wvaT = np.zeros((C, VW), dtype=np.float32)  # cast below
        for j in range(HL):
            wvaT[:, 65 * j : 65 * j + HD] = wv[HD * j : HD * j + HD, :].T
        projwT = np.ascontiguousarray(proj_w[:, cols].T).astype(bf16)
        in_maps.append(
            {
                "xT": xT,
                "wqT": wqT,
                "wkT": wkT,
                "wvaT": pack(wvaT.astype(bf16)),
                "cosT2": cosT2,
                "sinT2t": sinT2t,
                "projwT": projwT,
            }
        )
    return in_maps


def _install_ntff_hook():
    """Wire the axon NTFF profiling hook if the image's antenv lacks it."""
    import types

    try:
        from antenv.axon_hooks import get_axon_ntff_profile_hook  # noqa: F401

        return True
    except ImportError:
        pass
    try:
        import antenv
        from trn_agent_boot.trn_boot import _ntff_profile_via_ctypes

        hook = _ntff_profile_via_ctypes("/opt/axon/libaxon_pjrt.so")
        mod = types.ModuleType("antenv.axon_hooks")
        holder = {"hook": hook}
        mod.set_axon_ntff_profile_hook = lambda h: holder.__setitem__("hook", h)
        mod.get_axon_ntff_profile_hook = lambda: holder["hook"]
        sys.modules["antenv.axon_hooks"] = mod
        antenv.axon_hooks = mod
        return hook is not None
    except Exception as e:  # pragma: no cover
        print(f"ntff hook install failed: {e}")
        return False


_PROGRAM = None


def kernel(x, qkv_w, proj_w, proj_b):
    global _PROGRAM, LAST_RESULTS
    x = np.asarray(x, dtype=np.float32)
    qkv_w = np.asarray(qkv_w, dtype=np.float32)
    proj_w = np.asarray(proj_w, dtype=np.float32)
    proj_b = np.asarray(proj_b, dtype=np.float32)

    if _PROGRAM is None:
        _PROGRAM = _build_program()
    nc = _PROGRAM

    in_maps = _make_in_maps(x, qkv_w, proj_w)
    trace = bool(int(os.environ.get("KERNEL_TRACE", "0")))
    if trace:
        trace = _install_ntff_hook()
    res = run_bass_kernel_spmd(nc, in_maps, list(range(NCORES)), trace=trace)
    LAST_RESULTS = res

    out = np.empty((B, N, C), dtype=np.float32)
    for b in range(B):
        out[b] = res.results[G * b]["outp"].astype(np.float32) + res.results[
            G * b + 1
        ]["outp"].astype(np.float32)
    out += proj_b[None, None, :]
    return out


if __name__ == "__main__":
    x = np.random.randn(B, N, C).astype(np.float32)
    qkv_w = np.random.randn(3 * C, C).astype(np.float32)
    proj_w = np.random.randn(C, C).astype(np.float32)
    maps = _make_in_maps(x, qkv_w, proj_w)
    for k, v in maps[0].items():
        print(k, v.shape, v.dtype)


# revision 32
# speedup vs baseline: 1.1951x; 1.1951x over previous
"""Multi-head self-attention (B=4, N=2048, C=768, H=12, causal + RoPE) on 8 TRN2 cores.

Sharding: core = (batch b = core // 2, head-group g = core % 2); each core computes
6 heads of one batch end-to-end (qkv -> rope -> causal flash attention -> partial
output projection over its 384 channels). Host sums the two partial projections
per batch and adds the bias.

Device layout notes:
  - everything is kept "transposed" ([channel, token]); attention scores are
    computed directly as scoresT[k, q] = kT' . qT', PV needs no transposes.
  - phases are software-pipelined at every level: qkv for block b+1 and the
    output projection for block b-1 are emitted inside attention block b
    (after head-pair 1), exp runs TWO k-tiles ahead of PV, and each head
    pair's normalize is deferred into the next head pair's score stream.
  - qkv PSUM evacuation runs on the Act engine (idle during qkv); rope is
    2 muls + add on bf16 SBUF tiles, d-tile 0 on DVE (needed first), d-tiles
    1-2 on the otherwise-idle gpsimd.  swap32 is 4 partition-crossing SBUF
    DMAs per d-tile.
  - V carries an extra all-ones column per head; the PV matmul then
    accumulates the softmax denominator in psum row 64 for free.
  - causal trimming: on diagonal-band k-tiles only the valid q-suffix is
    computed; the in-band triangle is zeroed by a constant 0/1 bf16 mask
    multiply on DVE (NOT gpsimd affine_select: Pool dispatch latency of
    3-6us stalls the PV pipeline).
  - normalize: den rows copied to SBUF partitions 0 and 64 (engine start
    partitions must be 64-aligned), ONE K=65 selector matmul broadcasts
    1/denA to psum partitions 0-63 and 1/denB to 64-127, one reciprocal,
    then one stt per head writing attnT[0:64]/[64:128] directly (DVE
    operands may have different 64-aligned base partitions, so head B
    needs no shift DMA).  No gpsimd anywhere near the critical path.
  - input DMAs are split across the SP and Act hwdge queues (~85-150GB/s
    per queue), most-urgent (wq, x block 0) first; output DMAs of the last
    block alternate queues for the same reason.
  - PSUM budget: 2 banks qkv/V/proj/bcast ring + 4 banks score supers
    (double buffered) + 2 banks PV accumulators = 8.
"""

import os
import sys

import numpy as np

sys.path.insert(0, "/opt/trn_rl_repo")

import concourse.bass as bass
import concourse.mybir as mybir
import concourse.tile as tile
from concourse import bacc
from concourse.bass_utils import run_bass_kernel_spmd

dt = mybir.dt
F32 = dt.float32
BF16 = dt.bfloat16
AF = mybir.ActivationFunctionType
ALU = mybir.AluOpType

B, N, C = 4, 2048, 768
H, HD = 12, 64
HL = 6            # heads per core
G = 2             # head groups (cores per batch)
NCORES = 8
NT = N // 128     # 16 n-tiles
QB = 512          # query block
NQB = N // QB     # 4 query blocks
CT = C // 128     # 6 contraction tiles of x channels
DL = HL * HD      # 384 local channels
VW = HL * (HD + 1)  # 390: v columns + ones column per head
SCALE = float(HD) ** -0.5

LAST_RESULTS = None  # test harness can read exec_time_ns etc. from here


def _emit(nc, tc, dram):
    xT_d, wqT_d, wkT_d, wvaT_d, cosT2_d, sinT2t_d, projwT_d, outp_d = dram

    with tc.tile_pool(name="persist", bufs=1) as pp:
        qT = [pp.tile([128, N], BF16, tag=f"qT{t}", name=f"qT{t}") for t in range(3)]
        kT = [pp.tile([128, N], BF16, tag=f"kT{t}", name=f"kT{t}") for t in range(3)]
        attnT = [pp.tile([128, N], BF16, tag=f"aT{t}", name=f"aT{t}") for t in range(3)]
        V = [pp.tile([128, VW], BF16, tag=f"V{t}", name=f"V{t}") for t in range(NT)]

        wq = pp.tile([128, CT * DL], BF16, tag="wq", name="wq")
        wk = pp.tile([128, CT * DL], BF16, tag="wk", name="wk")
        wva = pp.tile([128, CT * VW], BF16, tag="wva", name="wva")
        cosT2 = pp.tile([128, N], BF16, tag="cosT2", name="cosT2")
        sinT2t = pp.tile([128, N], BF16, tag="sinT2t", name="sinT2t")
        pw = [pp.tile([128, C], BF16, tag=f"pw{t}", name=f"pw{t}") for t in range(3)]

        with (
            tc.tile_pool(name="mm_ps", bufs=2, space="PSUM") as mmp,
            tc.tile_pool(name="score_ps", bufs=2, space="PSUM") as scp,
            tc.tile_pool(name="out_ps", bufs=2, space="PSUM") as abp,
            tc.tile_pool(name="work", bufs=1) as ap,
        ):
            xtbs = {}
            pending = []  # deferred per-head-pair normalize closures

            def flush_pending():
                for fn in pending:
                    fn()
                pending.clear()

            def xtb_dma(nb):
                # two DMAs (SP + Act queues) load the 6 contraction tiles of
                # block nb: xtb[:, QB*ct + j] = x[token QB*nb+j, chan 128ct+p]
                xtb = ap.tile([128, CT * QB], BF16, tag="xtb", bufs=2, name=f"xtb{nb}")
                xv_ = xT_d[:].rearrange("p (c n) -> p c n", c=CT)
                nsl_ = slice(QB * nb, QB * (nb + 1))
                nc.sync.dma_start(xtb[:, 0 : 3 * QB], xv_[:, 0:3, nsl_])
                nc.scalar.dma_start(xtb[:, 3 * QB :], xv_[:, 3:6, nsl_])
                xtbs[nb] = xtb

            # --- preamble DMAs: wq/xtb0 halves split across SP+Act queues ---
            hw_ = CT * DL // 2
            nc.sync.dma_start(wq[:, 0:hw_], wqT_d[:, 0:hw_])
            nc.scalar.dma_start(wq[:, hw_:], wqT_d[:, hw_:])
            xtb_dma(0)
            nc.sync.dma_start(wk[:, 0:hw_], wkT_d[:, 0:hw_])
            nc.scalar.dma_start(wk[:, hw_:], wkT_d[:, hw_:])
            nc.sync.dma_start(cosT2[:], cosT2_d[:])
            nc.sync.dma_start(sinT2t[:], sinT2t_d[:])
            nc.sync.dma_start(wva[:], wvaT_d[:])
            for t in range(3):
                nc.sync.dma_start(pw[t][:], projwT_d[128 * t : 128 * (t + 1), :])
            # [65,128] selector for the one-matmul 1/den broadcast: row 0
            # routes denA to psum partitions 0-63, row 64 routes denB to
            # 64-127 (engine start-partitions must be 64-aligned, so denB
            # lives on partition 64; rows 1-63 are zero selectors).
            ones2 = pp.tile([65, 128], BF16, tag="ones2", name="ones2")
            nc.gpsimd.memset(ones2[:], 0.0)
            nc.gpsimd.memset(ones2[0:1, 0:64], 1.0)
            nc.gpsimd.memset(ones2[64:65, 64:128], 1.0)
            # pre-fill the den2 ring so its unused rows 1-63 are finite
            for i_ in range(3):
                d_ = ap.tile([65, QB], BF16, tag="den2", bufs=3, name=f"dninit{i_}")
                nc.gpsimd.memset(d_[:], 1.0)
            # constant 0/1 causal-triangle mask, duplicated for both heads:
            # tri2[p, 128h + j] = 1.0 if j >= p else 0.0
            tri2 = pp.tile([128, 256], BF16, tag="tri2", name="tri2")
            nc.gpsimd.memset(tri2[:], 1.0)
            nc.gpsimd.affine_select(
                out=tri2[:].rearrange("p (h f) -> p h f", h=2),
                in_=tri2[:].rearrange("p (h f) -> p h f", h=2),
                compare_op=ALU.is_ge,
                fill=0.0,
                base=0,
                pattern=[[0, 2], [1, 128]],
                channel_multiplier=-1,
            )

            def qkv_block(nb):
                nsl = slice(QB * nb, QB * (nb + 1))
                if nb not in xtbs:
                    xtb_dma(nb)
                xtb = xtbs[nb]
                if nb + 1 < NQB:
                    # prefetch next block's x right away (ring has room)
                    xtb_dma(nb + 1)
                qraw = ap.tile([128, 6 * QB], BF16, tag="qraw", bufs=2, name=f"qr{nb}")
                qswp = ap.tile([128, 6 * QB], BF16, tag="qswp", bufs=2, name=f"qs{nb}")
                # q/k matmuls, d-tile-major so pair 0's q and k arrive first;
                # PSUM evacuation on the (idle-during-qkv) Act engine
                for dtile in range(3):
                    for mi, (w, dest) in enumerate(((wq, qT), (wk, kT))):
                        ps = mmp.tile(
                            [128, QB], F32, tag="mm", name=f"ps_qk{nb}_{dtile}{mi}"
                        )
                        for ct in range(CT):
                            nc.tensor.matmul(
                                ps[:],
                                w[:, DL * ct + 128 * dtile : DL * ct + 128 * (dtile + 1)],
                                xtb[:, QB * ct : QB * (ct + 1)],
                                start=(ct == 0),
                                stop=(ct == CT - 1),
                            )
                        wcol = QB * (2 * dtile + mi)
                        nc.scalar.copy(qraw[:, wcol : wcol + QB], ps[:])
                    # swap32 for this dtile's q+k in 4 DMAs:
                    # qswp[64a+32b+c] = qraw[64a+32(1-b)+c]
                    dcol = QB * 2 * dtile
                    for blk in range(4):
                        lo = 32 * blk
                        swp = 32 * (blk + 1) if blk % 2 == 0 else 32 * (blk - 1)
                        nc.sync.dma_start(
                            qswp[lo : lo + 32, dcol : dcol + 2 * QB],
                            qraw[swp : swp + 32, dcol : dcol + 2 * QB],
                        )
                # rope: dst = qraw*cos + swap32(qraw)*sin_tau.  dtile 0 is
                # needed first (head pair 0) -> DVE; dtiles 1-2 have a full
                # head-pair of slack -> idle gpsimd.
                for dtile in range(3):
                    eng = nc.vector if dtile == 0 else nc.gpsimd
                    for mi, dest in enumerate((qT, kT)):
                        wcol = QB * (2 * dtile + mi)
                        dst = dest[dtile][:, nsl]
                        eng.tensor_mul(dst, qraw[:, wcol : wcol + QB], cosT2[:, nsl])
                        tmp = ap.tile(
                            [128, QB], BF16, tag="rtmp", bufs=4, name=f"rt{nb}_{dtile}{mi}"
                        )
                        eng.tensor_mul(
                            tmp[:], qswp[:, wcol : wcol + QB], sinT2t[:, nsl]
                        )
                        eng.tensor_add(dst, dst, tmp[:])
                # V for the 4 n-tiles of this block
                for sub in range(4):
                    nt = 4 * nb + sub
                    ps = mmp.tile([128, VW], F32, tag="mm", name=f"ps_v{nt}")
                    for ct in range(CT):
                        nc.tensor.matmul(
                            ps[:],
                            xtb[:, QB * ct + 128 * sub : QB * ct + 128 * (sub + 1)],
                            wva[:, VW * ct : VW * (ct + 1)],
                            start=(ct == 0),
                            stop=(ct == CT - 1),
                        )
                    nc.vector.tensor_copy(V[nt][:], ps[:])
                    ones_cols = V[nt][:].rearrange("p (h w) -> p h w", w=HD + 1)[
                        :, :, HD : HD + 1
                    ]
                    nc.gpsimd.memset(ones_cols, 1.0)

            def attention_block(qb):
                for pt in range(3):  # head pair (local heads 2pt, 2pt+1)
                    qsl = slice(QB * qb, QB * (qb + 1))
                    nkt = 4 * qb + 4  # causal: k-tiles 0 .. 4qb+3
                    psA = abp.tile([65, QB], F32, tag="outps", name=f"psA{pt}_{qb}")
                    psB = abp.tile([65, QB], F32, tag="outps", name=f"psB{pt}_{qb}")

                    def s_exp(kt):
                        # scoresT[k, q]: head A cols aa:512, B 512+aa:1024
                        ksl = slice(128 * kt, 128 * (kt + 1))
                        a = 128 * kt - QB * qb
                        aa = max(a, 0)
                        S = scp.tile(
                            [128, 2 * QB], F32, tag="sc", name=f"S{pt}_{qb}_{kt}"
                        )
                        for hh in range(2):
                            prow = slice(64 * hh, 64 * hh + 64)
                            nc.tensor.matmul(
                                S[:, QB * hh + aa : QB * (hh + 1)],
                                kT[pt][prow, ksl],
                                qT[pt][prow, QB * qb + aa : QB * (qb + 1)],
                                start=True,
                                stop=True,
                            )
                        P = ap.tile(
                            [128, 2 * QB], BF16, tag="probs", bufs=8,
                            name=f"P{pt}_{qb}_{kt}",
                        )
                        if a > 0:
                            Sv = S[:].rearrange("p (h q) -> p h q", h=2)[:, :, a:QB]
                            Pv = P[:].rearrange("p (h q) -> p h q", h=2)[:, :, a:QB]
                            nc.scalar.activation(Pv, Sv, AF.Exp, scale=SCALE)
                        else:
                            nc.scalar.activation(P[:], S[:], AF.Exp, scale=SCALE)
                        if a >= 0:
                            # zero the causal triangle of the diagonal band
                            # with a constant 0/1 mask on DVE (gpsimd dispatch
                            # latency would stall the PV pipeline)
                            Pband = P[:].rearrange("p (h q) -> p h q", h=2)[
                                :, :, a : a + 128
                            ]
                            nc.vector.tensor_mul(
                                Pband,
                                Pband,
                                tri2[:].rearrange("p (h f) -> p h f", h=2),
                            )
                        return P

                    def pv(kt, P):
                        aa = max(128 * kt - QB * qb, 0)
                        first, last = kt == 0, kt == nkt - 1
                        for hh, ps_out, h in ((0, psA, 2 * pt), (1, psB, 2 * pt + 1)):
                            nc.tensor.matmul(
                                ps_out[:, aa:QB],
                                V[kt][:, 65 * h : 65 * h + 65],
                                P[:, QB * hh + aa : QB * (hh + 1)],
                                start=first,
                                stop=last,
                                skip_group_check=True,
                            )

                    # software-pipelined: S/exp run TWO k-tiles ahead of PV so
                    # the Act engine's exp latency never stalls the PE
                    fifo = [s_exp(0), s_exp(1)]
                    flush_pending()
                    for kt in range(2, nkt):
                        fifo.append(s_exp(kt))
                        pv(kt - 2, fifo.pop(0))
                    pv(nkt - 2, fifo.pop(0))
                    pv(nkt - 1, fifo.pop(0))

                    # ---- normalize epilogue (decoupled from PSUM asap) ----
                    # No gpsimd here: the Pool engine's dispatch latency
                    # (3-6us) was stalling PV/proj.  1/den is broadcast to 64
                    # partitions by a tiny f32r rank-1 matmul instead.
                    rawA = ap.tile([64, QB], BF16, tag="rawA", bufs=3, name=f"rA{pt}_{qb}")
                    rawB = ap.tile([128, QB], BF16, tag="rawB", bufs=3, name=f"rB{pt}_{qb}")
                    den2 = ap.tile(
                        [65, QB], BF16, tag="den2", bufs=3, name=f"dn{pt}_{qb}"
                    )
                    nc.vector.tensor_copy(den2[0:1, :], psA[64:65, :])
                    nc.vector.tensor_copy(den2[64:65, :], psB[64:65, :])
                    nc.vector.tensor_copy(rawA[:], psA[0:64, :])
                    # head B raw goes straight to partitions 64-127 so the
                    # normalize can write attnT[64:128] without a shift DMA
                    nc.vector.tensor_copy(rawB[64:128, :], psB[0:64, :])

                    def normalize(pt=pt, qb=qb, qsl=qsl, rawA=rawA, rawB=rawB, den2=den2):
                        rbc = ap.tile(
                            [128, QB], F32, tag="rbc", bufs=2, name=f"rb{pt}_{qb}"
                        )
                        rb_ps = mmp.tile(
                            [128, QB], F32, tag="mm", name=f"rbp{pt}_{qb}"
                        )
                        nc.tensor.matmul(
                            rb_ps[:], ones2[:], den2[:], start=True, stop=True
                        )
                        nc.vector.reciprocal_approx_fast(rbc[:], rb_ps[:])
                        # the final scale has a full block of slack (proj for
                        # this qb runs one block later), so the otherwise-idle
                        # gpsimd can absorb it and shorten the DVE queue
                        nc.gpsimd.tensor_mul(
                            attnT[pt][0:64, qsl], rawA[:], rbc[0:64, :]
                        )
                        nc.gpsimd.tensor_mul(
                            attnT[pt][64:128, qsl], rawB[64:128, :], rbc[64:128, :]
                        )

                    pending.append(normalize)

                    if pt == 1:
                        # overlap next block's qkv and previous block's proj
                        # with the rest of this attention block
                        if qb + 1 < NQB:
                            qkv_block(qb + 1)
                        if qb > 0:
                            proj_block(qb - 1)

            def proj_block(b):
                for nt in range(4 * b, 4 * b + 4):
                    nsl = slice(128 * nt, 128 * (nt + 1))
                    osb = ap.tile([128, C], BF16, tag="osb", bufs=2, name=f"osb{nt}")
                    for half in range(2):
                        ps = mmp.tile([128, 384], F32, tag="mm", name=f"pj{half}_{nt}")
                        for ct in range(3):
                            nc.tensor.matmul(
                                ps[:],
                                attnT[ct][:, nsl],
                                pw[ct][:, 384 * half : 384 * (half + 1)],
                                start=(ct == 0),
                                stop=(ct == 2),
                                skip_group_check=True,
                            )
                        nc.vector.tensor_copy(
                            osb[:, 384 * half : 384 * (half + 1)], ps[:]
                        )
                    if b == NQB - 1 and nt % 2 == 1:
                        nc.scalar.dma_start(outp_d[nsl, :], osb[:])
                    else:
                        nc.sync.dma_start(outp_d[nsl, :], osb[:])

            qkv_block(0)
            for blk in range(NQB):
                attention_block(blk)
            flush_pending()
            proj_block(NQB - 1)


def _build_program():
    nc = bacc.Bacc(
        "TRN2",
        target_bir_lowering=False,
        debug=False,
        num_devices=NCORES,
    )

    dram = (
        nc.dram_tensor("xT", [128, CT * N], BF16, kind="ExternalInput"),
        nc.dram_tensor("wqT", [128, CT * DL], BF16, kind="ExternalInput"),
        nc.dram_tensor("wkT", [128, CT * DL], BF16, kind="ExternalInput"),
        nc.dram_tensor("wvaT", [128, CT * VW], BF16, kind="ExternalInput"),
        nc.dram_tensor("cosT2", [128, N], BF16, kind="ExternalInput"),
        nc.dram_tensor("sinT2t", [128, N], BF16, kind="ExternalInput"),
        nc.dram_tensor("projwT", [DL, C], BF16, kind="ExternalInput"),
        nc.dram_tensor("outp", [N, C], BF16, kind="ExternalOutput"),
    )

    with tile.TileContext(nc) as tc:
        _emit(nc, tc, dram)

    nc.compile()
    return nc


def _rope_tables():
    # mirror reference.rope_tables in float32 (keep the f32 product!)
    inv_freq = 1.0 / np.power(
        np.float32(10000.0), np.arange(0, HD, 2, dtype=np.float32) / np.float32(HD)
    )
    t = np.arange(N, dtype=np.float32)
    freqs = (t[:, None] * inv_freq[None, :].astype(np.float32)).astype(np.float32)
    emb = np.concatenate([freqs, freqs], axis=-1)  # [N, 64]
    return np.cos(emb).astype(np.float32), np.sin(emb).astype(np.float32)


def _make_in_maps(x, qkv_w, proj_w):
    import ml_dtypes

    bf16 = ml_dtypes.bfloat16
    cos, sin = _rope_tables()  # [N, 64]
    # cosT2[p, n] = cos[n, p % 64]
    dd = np.arange(128) % HD
    cosT2 = np.ascontiguousarray(cos.T[dd, :]).astype(bf16)  # [128, N]
    # sin_tau sign such that q' = q*cos + swap32(q)*sin_tau:
    # tau(d) = -1 for d%64 < 32, +1 otherwise  (swap happens BEFORE the
    # sin multiply now, so the sign sits at the destination index)
    sgn = np.where((dd % HD) < (HD // 2), np.float32(-1.0), np.float32(1.0))
    sinT2t = np.ascontiguousarray(sin.T[dd, :] * sgn[:, None]).astype(bf16)

    def pack(a):  # [768, w] -> [128, 6*w]: out[p, w*c + j] = a[128c + p, j]
        w = a.shape[1]
        return np.ascontiguousarray(
            a.reshape(CT, 128, w).transpose(1, 0, 2).reshape(128, CT * w)
        )

    in_maps = []
    for core in range(NCORES):
        b, g = core // G, core % G
        heads = [g * HL + j for j in range(HL)]
        cols = np.concatenate([np.arange(HD * h, HD * h + HD) for h in heads])
        xT = pack(np.ascontiguousarray(x[b].T)).astype(bf16)
        wqT = pack(np.ascontiguousarray(qkv_w[cols, :].T)).astype(bf16)
        wkT = pack(np.ascontiguousarray(qkv_w[C + cols, :].T)).astype(bf16)
        wv = qkv_w[2 * C + cols, :]  # [384, 768]
        wvaT = np.zeros((C, VW), dtype=np.float32)  # cast below
        for j in range(HL):
            wvaT[:, 65 * j : 65 * j + HD] = wv[HD * j : HD * j + HD, :].T
        projwT = np.ascontiguousarray(proj_w[:, cols].T).astype(bf16)
        in_maps.append(
            {
                "xT": xT,
                "wqT": wqT,
                "wkT": wkT,
                "wvaT": pack(wvaT.astype(bf16)),
                "cosT2": cosT2,
                "sinT2t": sinT2t,
                "projwT": projwT,
            }
        )
    return in_maps


def _install_ntff_hook():
    """Wire the axon NTFF profiling hook if the image's antenv lacks it."""
    import types

    try:
        from antenv.axon_hooks import get_axon_ntff_profile_hook  # noqa: F401

        return True
    except ImportError:
        pass
    try:
        import antenv
        from trn_agent_boot.trn_boot import _ntff_profile_via_ctypes

        hook = _ntff_profile_via_ctypes("/opt/axon/libaxon_pjrt.so")
        mod = types.ModuleType("antenv.axon_hooks")
        holder = {"hook": hook}
        mod.set_axon_ntff_profile_hook = lambda h: holder.__setitem__("hook", h)
        mod.get_axon_ntff_profile_hook = lambda: holder["hook"]
        sys.modules["antenv.axon_hooks"] = mod
        antenv.axon_hooks = mod
        return hook is not None
    except Exception as e:  # pragma: no cover
        print(f"ntff hook install failed: {e}")
        return False


_PROGRAM = None


def kernel(x, qkv_w, proj_w, proj_b):
    global _PROGRAM, LAST_RESULTS
    x = np.asarray(x, dtype=np.float32)
    qkv_w = np.asarray(qkv_w, dtype=np.float32)
    proj_w = np.asarray(proj_w, dtype=np.float32)
    proj_b = np.asarray(proj_b, dtype=np.float32)

    if _PROGRAM is None:
        _PROGRAM = _build_program()
    nc = _PROGRAM

    in_maps = _make_in_maps(x, qkv_w, proj_w)
    trace = bool(int(os.environ.get("KERNEL_TRACE", "0")))
    if trace:
        trace = _install_ntff_hook()
    res = run_bass_kernel_spmd(nc, in_maps, list(range(NCORES)), trace=trace)
    LAST_RESULTS = res

    out = np.empty((B, N, C), dtype=np.float32)
    for b in range(B):
        out[b] = res.results[G * b]["outp"].astype(np.float32) + res.results[
            G * b + 1
        ]["outp"].astype(np.float32)
    out += proj_b[None, None, :]
    return out


if __name__ == "__main__":
    x = np.random.randn(B, N, C).astype(np.float32)
    qkv_w = np.random.randn(3 * C, C).astype(np.float32)
    proj_w = np.random.randn(C, C).astype(np.float32)
    maps = _make_in_maps(x, qkv_w, proj_w)
    for k, v in maps[0].items():
        print(k, v.shape, v.dtype)
